# revision 21
# baseline (speedup 1.0000x reference)
"""Trainium2 Bass kernel for the Covid compartment forecast model.

Computation (per posterior sample s):
    growth[t,s] = r_t[t]**(1/T_serial[s]) * delta[s]
    A[t,s]      = A[t-1,s] * growth[t,s]            (scan, A[-1] = warmup[-1])
    A_full      = concat(warmup, A)                 # [J+T, S]
    M[t,s]      = sum_j A_full[J-1-j+t, s] * rho[s] * pi[j, s]

Sharding: posterior-sample dimension S across 8 cores (pure data parallel).
On-chip layout: samples on partitions (tiles of 128), time on the free dim.
warmup/pi are pre-transposed on the host to [S, J] so no on-chip transpose
is needed for them; PSUM is fully dedicated to the two accumulator pools.

Engine plan per 128-sample tile (24 PE taps / 8 tree taps, all ~95% busy):
  ACT : g = Exp(ln_r * invT + ln_delta) hoisted 2 tiles ahead; 8 scaled
        bf16 copies into the tree slab; PSUM->SBUF output copy
  DVE : A = tensor_tensor_scan(mult) in fp32 state -> bf16 A_full; all 24
        diag weight matrices in one bulk op (stride-0 broadcast views);
        pairwise-tree accumulation of the bf16 slab; final Mp+tree combine
  PE  : per-tap diag matmuls (bf16, 2x512-col chunks) accumulated in PSUM;
        8 f32r output transposes
All tap streams are bf16 (the fp32 scan state protects the recursion, so
only single-rounding errors enter each tap); host pre-transposes warmup/pi
and precomputes ln(r), ln(delta), 1/T_serial as input prep.
"""

import numpy as np

import concourse.bacc as bacc
import concourse.bass as bass
import concourse.mybir as mybir
import concourse.tile as tile
from concourse.bass_utils import run_bass_kernel_spmd

F32 = mybir.dt.float32
F32R = mybir.dt.float32r
BF16 = mybir.dt.bfloat16
I32 = mybir.dt.int32
AF = mybir.ActivationFunctionType
OP = mybir.AluOpType

T = 1024
J = 32
S_TOTAL = 50000
NCORES = 8
P = 128
S_CORE = S_TOTAL // NCORES           # 6250
NTILES = (S_CORE + P - 1) // P       # 49
S_PAD = NTILES * P                   # 6272

# Tap routing. PE taps go through diag-weight matmuls into PSUM (all diag
# matrices built in one bulk DVE op); tree taps are scaled into a bf16 slab
# and pairwise-tree reduced on DVE.
N_PE = 24                 # taps 0..23 on PE
N_TREE = J - N_PE         # taps 24..31 via the slab tree
N_ACT_SCALE = 8           # of the tree taps, how many scaled copies ACT does


def build(n_pe=N_PE, n_act_scale=N_ACT_SCALE):
    n_tree = J - n_pe
    assert 1 <= n_pe <= J
    assert 0 <= n_act_scale <= n_tree

    nc = bacc.Bacc("TRN2", target_bir_lowering=False, debug=False,
                   num_devices=NCORES)
    r = nc.dram_tensor("ln_r", [1, T], F32, kind="ExternalInput").ap()
    wu = nc.dram_tensor("warmup_t", [S_PAD, J], F32, kind="ExternalInput").ap()
    pi = nc.dram_tensor("pi_t", [S_PAD, J], F32, kind="ExternalInput").ap()
    dl = nc.dram_tensor("ln_delta", [1, S_PAD], F32, kind="ExternalInput").ap()
    ts = nc.dram_tensor("inv_ts", [1, S_PAD], F32, kind="ExternalInput").ap()
    rh = nc.dram_tensor("rho", [1, S_PAD], F32, kind="ExternalInput").ap()
    m = nc.dram_tensor("m_out", [T, S_PAD], F32, kind="ExternalOutput").ap()

    # [1, S_PAD] DRAM param -> [P, NTILES] SBUF layout: (p, i) = param[i*P + p]
    def param_ap(a):
        return bass.AP(tensor=a.tensor, offset=a.offset,
                       ap=[[1, P], [P, NTILES]])

    with tile.TileContext(nc) as tc:
        with (
            tc.tile_pool(name="singles", bufs=1) as singles,
            tc.tile_pool(name="loads", bufs=3) as loads,
            tc.tile_pool(name="apool", bufs=2) as apool,
            tc.tile_pool(name="gpool", bufs=3) as gpool,
            tc.tile_pool(name="qpool", bufs=3) as qpool,
            tc.tile_pool(name="diags", bufs=2) as diags,
            tc.tile_pool(name="slab", bufs=2) as slabp,
            tc.tile_pool(name="msb", bufs=2) as msb,
            tc.tile_pool(name="mtsb", bufs=2) as mtsb,
            tc.tile_pool(name="mpsum", bufs=2, space="PSUM") as mpsum,
            tc.tile_pool(name="mtpsum", bufs=2, space="PSUM") as mtpsum,
        ):
            # ---- one-time setup: params land precomputed from the host ----
            lnd_all = singles.tile([P, NTILES], F32)
            nc.sync.dma_start(out=lnd_all, in_=param_ap(dl))
            invT_all = singles.tile([P, NTILES], F32)
            nc.sync.dma_start(out=invT_all, in_=param_ap(ts))
            rh_sb = singles.tile([P, NTILES], F32)
            nc.sync.dma_start(out=rh_sb, in_=param_ap(rh))
            lr_bc = singles.tile([P, T], F32)
            nc.sync.dma_start(
                out=lr_bc,
                in_=bass.AP(tensor=r.tensor, offset=r.offset,
                            ap=[[0, P], [1, T]]))

            iota_t = singles.tile([P, P], I32)
            nc.gpsimd.iota(iota_t, pattern=[[1, P]], base=0,
                           channel_multiplier=-1)
            identb = singles.tile([P, P], BF16)
            nc.vector.tensor_scalar(out=identb, in0=iota_t, scalar1=0,
                                    scalar2=None, op0=OP.is_equal)
            ident_r = singles.tile([P, P], F32R)
            nc.vector.tensor_scalar(out=ident_r, in0=iota_t, scalar1=0,
                                    scalar2=None, op0=OP.is_equal)

            # ---- per sample-tile, software-pipelined ----
            # Stages are emitted so each engine queue stays dense:
            # ACT does tile i+1's Exp while DVE runs tile i's scan; DVE runs
            # tile i-1's tree/combine after issuing tile i's scan.
            gq = {}
            state = {}

            def fexp(i):
                g = gpool.tile([P, T], F32)
                nc.scalar.activation(out=g, in_=lr_bc, func=AF.Exp,
                                     bias=lnd_all[:, i:i + 1],
                                     scale=invT_all[:, i:i + 1])
                gq[i] = g

            def front(i):
                s0 = i * P

                wuT = loads.tile([P, J], F32, tag="wu")
                nc.sync.dma_start(out=wuT, in_=wu[s0:s0 + P, :])
                piT = loads.tile([P, J], F32, tag="pi")
                nc.sync.dma_start(out=piT, in_=pi[s0:s0 + P, :])

                q = qpool.tile([P, J], F32, tag="q")
                nc.vector.tensor_scalar(out=q, in0=piT,
                                        scalar1=rh_sb[:, i:i + 1],
                                        scalar2=None, op0=OP.mult)
                qb = qpool.tile([P, J], BF16, tag="qb")
                nc.vector.tensor_copy(out=qb, in_=q)

                A_full = apool.tile([P, J + T], BF16)
                nc.vector.tensor_copy(out=A_full[:, 0:J], in_=wuT)

                g = gq.pop(i)

                # A[t] = A[t-1] * g[t]; fp32 internal state, bf16 output
                nc.vector.tensor_tensor_scan(
                    out=A_full[:, J:J + T], data0=g, data1=g,
                    initial=wuT[:, J - 1:J], op0=OP.mult, op1=OP.bypass)

                # all n_pe diag matrices in one op
                dslab = diags.tile([P, n_pe * P], BF16)
                nc.vector.tensor_tensor(
                    out=dslab,
                    in0=bass.AP(tensor=identb.tensor, offset=identb.offset,
                                ap=[[P, P], [0, n_pe], [1, P]]),
                    in1=bass.AP(tensor=qb.tensor, offset=qb.offset,
                                ap=[[J, P], [1, n_pe], [0, P]]),
                    op=OP.mult)

                def shift(j):
                    return A_full[:, J - 1 - j:J - 1 - j + T]

                # ---- PE taps: diag(q_j) matmuls accumulated in PSUM ----
                Mp = mpsum.tile([P, T], F32, tag="Mp")
                for k in range(n_pe):
                    dg = dslab[:, k * P:(k + 1) * P]
                    last = k == n_pe - 1
                    for c in range(2):
                        lo = c * 512
                        nc.tensor.matmul(Mp[:, lo:lo + 512], dg,
                                         shift(k)[:, lo:lo + 512],
                                         start=(k == 0), stop=last)

                # ---- tree taps: scaled bf16 copies into the slab ----
                slab = slabp.tile([P, n_tree * T], BF16)
                for kk in range(n_tree):
                    j = n_pe + kk
                    dst = slab[:, kk * T:(kk + 1) * T]
                    if kk < n_act_scale:
                        nc.scalar.activation(out=dst, in_=shift(j),
                                             func=AF.Copy,
                                             scale=q[:, j:j + 1])
                    else:
                        nc.vector.tensor_scalar(out=dst, in0=shift(j),
                                                scalar1=q[:, j:j + 1],
                                                scalar2=None, op0=OP.mult)
                state[i] = (slab, Mp)

            def back(i):
                s0 = i * P
                slab, Mp = state.pop(i)
                width = n_tree
                while width > 1:
                    half = width // 2
                    odd = width - 2 * half
                    nc.vector.tensor_tensor(
                        out=slab[:, 0:half * T],
                        in0=slab[:, 0:half * T],
                        in1=slab[:, half * T:2 * half * T],
                        op=OP.add)
                    if odd:
                        # fold the odd slab into slot 0 next round
                        nc.vector.tensor_tensor(
                            out=slab[:, 0:T],
                            in0=slab[:, 0:T],
                            in1=slab[:, (width - 1) * T:width * T],
                            op=OP.add)
                    width = half

                # ---- combine + output transpose ----
                M_sb = msb.tile([P, T], F32R)
                nc.vector.tensor_tensor(out=M_sb, in0=Mp,
                                        in1=slab[:, 0:T], op=OP.add)

                Mt_ps = mtpsum.tile([P, T], F32R, tag="mt")
                for b in range(T // P):
                    nc.tensor.transpose(Mt_ps[:, b * P:(b + 1) * P],
                                        M_sb[:, b * P:(b + 1) * P], ident_r)
                Mt_sb = mtsb.tile([P, T], F32)
                nc.scalar.copy(out=Mt_sb, in_=Mt_ps)

                # Mt_sb[:, b*P:(b+1)*P] holds M rows [b*P, (b+1)*P) of this
                # sample block; one strided DMA scatters all 8 blocks.
                out_ap = bass.AP(tensor=m.tensor, offset=m.offset + s0,
                                 ap=[[S_PAD, P], [P * S_PAD, T // P], [1, P]])
                nc.sync.dma_start(out=out_ap, in_=Mt_sb)

            fexp(0)
            fexp(1)
            fexp(2)
            front(0)
            for i in range(1, NTILES):
                if i + 2 < NTILES:
                    fexp(i + 2)
                front(i)
                back(i - 1)
            back(NTILES - 1)

    nc.compile()
    return nc


_NC_CACHE = {}


def _get_nc(key=(N_PE, N_ACT_SCALE)):
    if key not in _NC_CACHE:
        _NC_CACHE[key] = build(*key)
    return _NC_CACHE[key]


def _shard_inputs(r_t, warmup_A, delta, T_serial, rho_M, pi_M):
    """Slice the full inputs per core, pad S_CORE -> S_PAD, transpose
    warmup/pi to sample-major so the kernel needs no on-chip transpose."""
    lnr = np.log(np.asarray(r_t, dtype=np.float64)).astype(np.float32).reshape(1, T)
    wu_t = np.ascontiguousarray(np.asarray(warmup_A, dtype=np.float32).T)
    pi_t = np.ascontiguousarray(np.asarray(pi_M, dtype=np.float32).T)
    in_maps = []
    for c in range(NCORES):
        lo, hi = c * S_CORE, (c + 1) * S_CORE
        pad = S_PAD - S_CORE

        def pad2t(a, fill):
            return np.pad(a[lo:hi], ((0, pad), (0, 0)), constant_values=fill)

        def pad1(a, fill):
            a = np.asarray(a, dtype=np.float32)[lo:hi].reshape(1, -1)
            return np.pad(a, ((0, 0), (0, pad)), constant_values=fill)

        in_maps.append({
            "ln_r": lnr,
            "warmup_t": pad2t(wu_t, 1.0),
            "pi_t": pad2t(pi_t, 1.0 / J),
            "ln_delta": pad1(np.log(np.asarray(delta, dtype=np.float64)), 0.0),
            "inv_ts": pad1(1.0 / np.asarray(T_serial, dtype=np.float64), 0.2),
            "rho": pad1(rho_M, 0.0),
        })
    return in_maps


def run(inputs, trace=False, key=(N_PE, N_ACT_SCALE), **kwargs):
    """Run on 8 cores; returns (M [T, S_TOTAL] float32, BassKernelResults)."""
    nc = _get_nc(key)
    in_maps = _shard_inputs(**inputs)
    res = run_bass_kernel_spmd(nc, in_maps, core_ids=list(range(NCORES)),
                               trace=trace, **kwargs)
    M = np.concatenate(
        [res.results[c]["m_out"][:, :S_CORE] for c in range(NCORES)], axis=1)
    return np.ascontiguousarray(M, dtype=np.float32), res


def kernel(**inputs):
    M, _ = run(inputs)
    return M
